# revision 41
# baseline (speedup 1.0000x reference)
"""Trainium2 Bass kernel for dist-biased multi-head attention.

Reference computation (jax):
    qkv = x @ w_qkv; q,k,v = split(qkv); heads of 64
    dots = einsum('bhnd,bhmd->bhnm', q, k) * scale + dist
    attn = softmax(dots, axis=-1)
    out  = einsum('bhnm,bhmd->bhnd', attn, v) -> merge heads -> @ w_out + b_out

Shapes: x [4, 2048, 512], dist [4, 8, 2048, 2048], w_qkv [512, 1536],
w_out [512, 512], b_out [512].

Sharding over 8 cores: core m handles batch m//2, heads 4*(m%2) .. +4.
Each core computes its 4 heads' attention plus the partial out-projection
for its batch; host sums the two partials per batch and adds b_out.

Device layout (v2 — head-PAIR stacking on the partition axis):
 - scores are computed TRANSPOSED: S^T [keys(part), queries(free)] so the
   attn@v matmul contracts keys on the partition dim with no transposes.
 - q^T/k^T for a head PAIR are stacked on partitions: rows 0-63 = head 2p,
   rows 64-127 = head 2p+1. The two heads' QK^T matmuls are K=64 row-tiles
   at array rows (0,0)/(64,0) and run CONCURRENTLY in the PE (2x QK rate);
   the q/k projections become full M=128 matmuls (2x projection rate); the
   out-projection contracts the stacked pair K=128 (2x rate, PSUM
   accumulates across pairs).
 - dist is folded in as exp(dots) = exp(qk)*exp(dist): host precomputes
   expd = exp(dist^T) in bf16; the device multiplies it into exp(scores)
   on the DVE (bf16 2x mode), keeping the PE free of the identity-matmul
   dist-add.  Variant "ped": dist^T added on the PE instead (identity
   matmul accumulation, as v1).
 - softmax skips the max-subtraction (scores are O(10); exp stays inside
   fp32/bf16 range) and the denominator comes from augmenting v with a
   ones column (row 64 of the AV output).
 - attention stage in bf16 (q/k/v/exp/expd); projections in float32r
   (full PE rate for free dim >= 256); all PSUM accumulation fp32.
 - engine budget per core: ACT ~147us (exp only), PE ~110us, DVE ~105us
   (expd mul + evacuations + normalize), DMA ~100us (expd bf16 32MiB).
"""

import numpy as np

N_CORES = 8
B = 4
NTOK = 2048
DIM = 512
HEADS = 8
DH = 64  # head dim
NH = HEADS // 2  # heads per core (4)
NPAIR = NH // 2  # head pairs per core (2)
INNER = HEADS * DH
SCALE = DH ** -0.5
QC = 1024  # query chunk (free-dim) per attention psum block
NKB = NTOK // 128  # key blocks of 128
# key blocks whose dist bias is added on the PE (identity matmul accumulate);
# the rest multiply host-precomputed exp(dist) on the DVE. The PE share keeps
# the tensor engine ~100% busy during attention (HAM stays at K=8/8) while
# ACT (exp) remains the binding engine.
PE_KB = tuple(kb for kb in range(NKB) if kb % 3 == 0)


def _kb_on_pe(variant, kb):
    if "ped" in variant:
        return True
    if "mix3" in variant:
        return kb in PE_KB
    return False


def _build_nc(repeats=1, variant="full"):
    """variant flags (substring match):
      full  - default: host exp(dist), DVE multiply
      ped   - dist^T added on the PE via identity matmuls (no host exp)
      timing-only ablations (results wrong): nodma (skip dist DMA),
      nomul (skip the DVE multiply / PE dist-add), noav (skip attn@v).
    """
    import concourse.bacc as bacc
    import concourse.mybir as mybir
    import concourse.tile as tile
    from concourse.bass import ts
    from concourse.masks import make_identity

    f32 = mybir.dt.float32
    bf16 = mybir.dt.bfloat16
    Exp = mybir.ActivationFunctionType.Exp

    ped = "ped" in variant
    dbg = "dbg" in variant

    nc = bacc.Bacc("TRN2", target_bir_lowering=False, debug=False)

    xT_d = nc.dram_tensor("xT", [DIM, NTOK], bf16, kind="ExternalInput").ap()
    wq_d = nc.dram_tensor("wq", [DIM, NH * DH], bf16, kind="ExternalInput").ap()
    wk_d = nc.dram_tensor("wk", [DIM, NH * DH], bf16, kind="ExternalInput").ap()
    wv_d = nc.dram_tensor("wv", [DIM, NH * DH], bf16, kind="ExternalInput").ap()
    # expdT ("full"): exp(dist^T); distT ("ped"): dist^T
    ed_d = nc.dram_tensor("expdT", [NH, NTOK, NTOK], bf16, kind="ExternalInput").ap()
    wo_d = nc.dram_tensor("wo", [NH * DH, DIM], bf16, kind="ExternalInput").ap()
    part_d = nc.dram_tensor("part", [NTOK, DIM], f32, kind="ExternalOutput").ap()
    if dbg:
        qT2_o = nc.dram_tensor("qT2_o", [128, NPAIR, NTOK], bf16, kind="ExternalOutput").ap()
        kT2_o = nc.dram_tensor("kT2_o", [128, NPAIR, NTOK], bf16, kind="ExternalOutput").ap()
        v_o = nc.dram_tensor("v_o", [128, NH, NKB, DH + 1], bf16, kind="ExternalOutput").ap()
        ex_o = nc.dram_tensor("ex_o", [128, QC], bf16, kind="ExternalOutput").ap()
        em_o = nc.dram_tensor("em_o", [128, QC], bf16, kind="ExternalOutput").ap()
        po_o = nc.dram_tensor("po_o", [DH + 1, QC], f32, kind="ExternalOutput").ap()
        rec_o = nc.dram_tensor("rec_o", [1, QC], f32, kind="ExternalOutput").ap()
        oT2_o = nc.dram_tensor("oT2_o", [128, QC], bf16, kind="ExternalOutput").ap()

    with tile.TileContext(nc) as tc:
        for _rep in range(repeats):
            with (
                tc.tile_pool(name="consts", bufs=1) as consts,
                tc.tile_pool(name="qkv", bufs=1) as qkv,
            ):
                xT_sb = consts.tile([128, DIM // 128, NTOK], bf16)
                nc.sync.dma_start(xT_sb[:], xT_d.rearrange("(c p) n -> p c n", p=128))
                wq_sb = consts.tile([128, DIM // 128, NH * DH], bf16)
                nc.sync.dma_start(wq_sb[:], wq_d.rearrange("(c p) n -> p c n", p=128))
                wk_sb = consts.tile([128, DIM // 128, NH * DH], bf16)
                nc.sync.dma_start(wk_sb[:], wk_d.rearrange("(c p) n -> p c n", p=128))
                wv_sb = consts.tile([128, DIM // 128, NH * DH], bf16)
                nc.sync.dma_start(wv_sb[:], wv_d.rearrange("(c p) n -> p c n", p=128))
                # w_out rows for the pair stacked on partitions: [128, pair, DIM]
                wo_sb = consts.tile([128, NPAIR, DIM], bf16)
                nc.sync.dma_start(wo_sb[:], wo_d.rearrange("(p q) n -> q p n", q=128))

                ident32 = consts.tile([128, 128], f32)
                make_identity(nc, ident32)
                ident = consts.tile([128, 128], bf16)
                nc.scalar.copy(ident[:], ident32[:])

                # head pair p stacked on partitions: rows 0-63 head 2p, 64-127 head 2p+1
                qT2 = qkv.tile([128, NPAIR, NTOK], bf16)
                kT2 = qkv.tile([128, NPAIR, NTOK], bf16)
                v_sb = qkv.tile([128, NH, NKB, DH + 1], bf16)
                ones32 = consts.tile([128, NH, NKB, 1], f32)
                nc.gpsimd.memset(ones32[:], 1.0)
                nc.scalar.copy(v_sb[:, :, :, DH : DH + 1], ones32[:])

                # ---- unified pipeline: minimal warmup projections, then
                # attention with v/k/q projections + out-projections
                # interleaved so the PE never drains ----
                with (
                    tc.tile_pool(name="spsum", bufs=2, space="PSUM") as spsum,
                    tc.tile_pool(name="opsum", bufs=2, space="PSUM") as opsum,
                    tc.tile_pool(name="distp", bufs=10) as distp,
                    tc.tile_pool(name="expp", bufs=10) as expp,
                    tc.tile_pool(name="emp", bufs=8) as emp,
                    tc.tile_pool(name="otp", bufs=4) as otp,
                    tc.tile_pool(name="pocp", bufs=4) as pocp,
                    tc.tile_pool(name="smalls", bufs=4) as smalls,
                    tc.tile_pool(name="outp", bufs=3) as outp,
                ):
                    def qk_proj(dst, w_sb, p, half, act_evac):
                        ps = spsum.tile([128, QC], f32, name="ps", tag="ps")
                        for c in range(DIM // 128):
                            for j in range(QC // 512):
                                nc.tensor.matmul(
                                    ps[:, ts(j, 512)],
                                    w_sb[:, c, ts(p, 128)],
                                    xT_sb[:, c, half * QC + 512 * j : half * QC + 512 * (j + 1)],
                                    start=(c == 0),
                                    stop=(c == DIM // 128 - 1),
                                )
                        if act_evac:
                            nc.scalar.copy(dst[:, p, ts(half, QC)], ps[:])
                        else:
                            nc.vector.tensor_copy(dst[:, p, ts(half, QC)], ps[:])

                    def v_proj(i, act_evac):
                        ps_v = spsum.tile([128, NH * DH], f32, name="ps_v", tag="ps")
                        for c in range(DIM // 128):
                            nc.tensor.matmul(
                                ps_v[:],
                                xT_sb[:, c, ts(i, 128)],
                                wv_sb[:, c, :],
                                start=(c == 0),
                                stop=(c == DIM // 128 - 1),
                            )
                        ev = nc.scalar.copy if act_evac else nc.vector.tensor_copy
                        ev(v_sb[:, :, i, 0:DH], ps_v.rearrange("p (h d) -> p h d", h=NH))

                    def out_proj(oT2_src, qc_src, i, act_evac=False):
                        # pair-stacked K=128, accumulate pairs in PSUM
                        pp = spsum.tile([128, QC], f32, name="pp", tag="ps")
                        for p in range(NPAIR):
                            nc.tensor.matmul(
                                pp[:, 0:DIM],
                                oT2_src[p][:, ts(i, 128)],
                                wo_sb[:, p, :],
                                start=(p == 0),
                                stop=(p == NPAIR - 1),
                            )
                        ob = outp.tile([128, DIM], f32, name="ob")
                        if act_evac:
                            nc.scalar.copy(ob[:], pp[:, 0:DIM])
                        else:
                            nc.vector.tensor_copy(ob[:], pp[:, 0:DIM])
                        nc.sync.dma_start(
                            part_d[qc_src * QC + i * 128 : qc_src * QC + (i + 1) * 128, :],
                            ob[:],
                        )

                    # warmup: just enough for (qc0, pair0) to start; the rest
                    # interleaves into its kb loop below
                    qk_proj(kT2, wk_sb, 0, 0, act_evac=True)
                    qk_proj(kT2, wk_sb, 0, 1, act_evac=True)
                    qk_proj(qT2, wq_sb, 0, 0, act_evac=True)

                    # (emitted at (p==0, kb): deferred projection work; ACT
                    # evacs — during qc0 p0 the PE, not ACT, is the pacer)
                    qc0_extra = {
                        4: lambda: qk_proj(kT2, wk_sb, 1, 0, True),
                        6: lambda: qk_proj(kT2, wk_sb, 1, 1, True),
                        8: lambda: qk_proj(qT2, wq_sb, 1, 0, True),
                        10: lambda: qk_proj(qT2, wq_sb, 0, 1, True),
                        12: lambda: qk_proj(qT2, wq_sb, 1, 1, True),
                    }

                    prev_oT2 = None
                    for qc in range(NTOK // QC):
                        oT2 = [otp.tile([128, QC], bf16, name="oT2") for _ in range(NPAIR)]
                        for p in range(NPAIR):
                            po = [opsum.tile([DH + 1, QC], f32, name="po") for _ in range(2)]
                            for kb in range(NKB):
                                ed_t = []
                                for s in range(2):
                                    t = distp.tile([128, QC], bf16, name="ed")
                                    if "nodma" not in variant:
                                        nc.sync.dma_start(
                                            t[:], ed_d[2 * p + s, ts(kb, 128), ts(qc, QC)]
                                        )
                                    ed_t.append(t)
                                # interleaved deferred projections must precede
                                # the AV that first reads them (PE is in-order)
                                if qc == 0 and p == 0:
                                    v_proj(kb, act_evac=True)
                                    if kb in qc0_extra:
                                        qc0_extra[kb]()
                                on_pe = _kb_on_pe(variant, kb) and "nomul" not in variant
                                ps = [spsum.tile([128, QC], f32, name="ps") for _ in range(2)]
                                # two K=64 row-tiles (rows 0-63 / 64-127), issued
                                # alternating (a,b,a,b) so adjacent MMs target
                                # disjoint row groups and run concurrently, and
                                # each LDWEIGHTS hits rows the streaming MM
                                # doesn't occupy
                                for s, j in ((0, 0), (1, 0), (0, 1), (1, 1)):
                                    pb = 64 * s
                                    nc.tensor.matmul(
                                        ps[s][:, ts(j, 512)],
                                        kT2[pb : pb + 64, p, ts(kb, 128)],
                                        qT2[pb : pb + 64, p, qc * QC + 512 * j : qc * QC + 512 * (j + 1)],
                                        start=True,
                                        stop=not on_pe,
                                    )
                                if on_pe:
                                    for s in range(2):
                                        for j in range(QC // 512):
                                            nc.tensor.matmul(
                                                ps[s][:, ts(j, 512)],
                                                ident[:],
                                                ed_t[s][:, ts(j, 512)],
                                                start=False,
                                                stop=True,
                                            )
                                for s in range(2):
                                    ex = expp.tile([128, QC], bf16, name="ex")
                                    nc.scalar.activation(ex[:], ps[s][:], Exp)
                                    if on_pe or "nomul" in variant:
                                        em = ex
                                    else:
                                        em = emp.tile([128, QC], bf16, name="em")
                                        nc.vector.tensor_mul(em[:], ex[:], ed_t[s][:])
                                    if dbg and qc == 0 and p == 0 and kb == 0 and s == 0:
                                        nc.sync.dma_start(ex_o[:], ex[:])
                                        nc.sync.dma_start(em_o[:], em[:])
                                    if "noav" not in variant:
                                        for j in range(QC // 512):
                                            nc.tensor.matmul(
                                                po[s][:, ts(j, 512)],
                                                v_sb[:, 2 * p + s, kb, :],
                                                em[:, ts(j, 512)],
                                                start=(kb == 0),
                                                stop=(kb == NKB - 1),
                                            )
                                # previous chunk's out-projection, interleaved so
                                # the PE never idles at the qc boundary
                                if prev_oT2 is not None and p == 0 and kb % 2 == 1:
                                    out_proj(prev_oT2, qc - 1, kb // 2)
                            # normalize (softmax denom = row 64): first copy po
                            # out of PSUM (frees the opsum ring for the next
                            # pair), then reciprocal/broadcast/scale from SBUF
                            for s in range(2):
                                poc = pocp.tile([DH, QC], f32, name="poc")
                                nc.vector.tensor_copy(poc[:], po[s][0:DH, :])
                                den = smalls.tile([1, QC], f32, name="den", tag="den")
                                nc.vector.tensor_copy(den[:], po[s][DH : DH + 1, :])
                                # NOTE: reciprocal_approx_fast requires a
                                # base-partition-0 input (base-64 views return
                                # garbage), hence the den copy
                                rec = smalls.tile([1, QC], f32, name="rec", tag="rec")
                                nc.vector.reciprocal_approx_fast(rec[:], den[:])
                                rb = smalls.tile([DH, QC], f32, name="rb", tag="rb")
                                nc.gpsimd.partition_broadcast(rb[:], rec[:])
                                # on gpsimd (idle) to keep the DVE free for the
                                # exp(dist) multiplies
                                nc.gpsimd.tensor_mul(
                                    oT2[p][64 * s : 64 * s + 64, :], poc[:], rb[:]
                                )
                                if dbg and qc == 0 and p == 0 and s == 0:
                                    nc.sync.dma_start(po_o[0:DH, :], poc[:])
                                    nc.sync.dma_start(po_o[DH : DH + 1, :], den[:])
                                    nc.sync.dma_start(rec_o[:], rec[:])
                            if dbg and qc == 0 and p == 0:
                                nc.sync.dma_start(oT2_o[:], oT2[0][:])
                        prev_oT2 = oT2
                    # last chunk's out-projection (tail; ACT is idle here)
                    for i in range(QC // 128):
                        out_proj(prev_oT2, NTOK // QC - 1, i, act_evac=True)
                    if dbg:
                        nc.sync.dma_start(qT2_o[:], qT2[:])
                        nc.sync.dma_start(kT2_o[:], kT2[:])
                        nc.sync.dma_start(v_o[:], v_sb[:])

    nc.compile()
    return nc


_NC_CACHE = {}


def _get_nc(repeats=1, variant=None):
    if variant is None:
        variant = KERNEL_VARIANT
    key = (repeats, variant)
    if key not in _NC_CACHE:
        _NC_CACHE[key] = _build_nc(repeats, variant)
    return _NC_CACHE[key]


def make_in_maps(x, dist, w_qkv, w_out, variant=None):
    """Host-side sharding: per-core input dicts (final device dtypes)."""
    import ml_dtypes

    if variant is None:
        variant = KERNEL_VARIANT
    bf16 = ml_dtypes.bfloat16
    x = np.asarray(x, dtype=np.float32)
    dist = np.asarray(dist, dtype=np.float32)
    w_qkv = np.asarray(w_qkv, dtype=np.float32)
    w_out = np.asarray(w_out, dtype=np.float32)
    in_maps = []
    for m in range(N_CORES):
        b = m // 2
        h0 = NH * (m % 2)
        wq = np.ascontiguousarray(w_qkv[:, h0 * DH : (h0 + NH) * DH]) * np.float32(SCALE)
        wk = np.ascontiguousarray(w_qkv[:, INNER + h0 * DH : INNER + (h0 + NH) * DH])
        wv = np.ascontiguousarray(w_qkv[:, 2 * INNER + h0 * DH : 2 * INNER + (h0 + NH) * DH])
        dT = np.ascontiguousarray(dist[b, h0 : h0 + NH].transpose(0, 2, 1))
        if "ped" in variant:
            ed = dT.astype(bf16)
        elif "mix3" in variant:
            # mixed: PE_KB key blocks carry raw dist (PE identity-add), the
            # rest carry exp(dist) (DVE multiply)
            ed = np.exp(dT)
            for kb in PE_KB:
                ed[:, kb * 128 : (kb + 1) * 128, :] = dT[:, kb * 128 : (kb + 1) * 128, :]
            ed = ed.astype(bf16)
        else:
            ed = np.exp(dT).astype(bf16)
        in_maps.append(
            {
                "xT": np.ascontiguousarray(x[b].T).astype(bf16),
                "wq": wq.astype(bf16),
                "wk": wk.astype(bf16),
                "wv": wv.astype(bf16),
                "expdT": ed,
                "wo": np.ascontiguousarray(w_out[h0 * DH : (h0 + NH) * DH, :]).astype(bf16),
            }
        )
    return in_maps


def cast_in_maps(nc, in_maps):
    """Compat shim: make_in_maps already emits final dtypes."""
    return in_maps


def assemble(results, b_out):
    """Sum the two per-batch partials and add bias."""
    out = np.empty((B, NTOK, DIM), dtype=np.float32)
    for b in range(B):
        out[b] = results[2 * b]["part"] + results[2 * b + 1]["part"] + b_out
    return out


KERNEL_VARIANT = "mix3"


def kernel(x, dist, w_qkv, w_out, b_out):
    from concourse.bass_utils import run_bass_kernel_spmd

    nc = _get_nc(variant=KERNEL_VARIANT)
    in_maps = make_in_maps(x, dist, w_qkv, w_out)
    res = run_bass_kernel_spmd(nc, in_maps, core_ids=list(range(N_CORES)))
    return assemble(res.results, np.asarray(b_out, dtype=np.float32))


# revision 42
# speedup vs baseline: 1.1927x; 1.1927x over previous
"""Trainium2 Bass kernel for dist-biased multi-head attention.

Reference computation (jax):
    qkv = x @ w_qkv; q,k,v = split(qkv); heads of 64
    dots = einsum('bhnd,bhmd->bhnm', q, k) * scale + dist
    attn = softmax(dots, axis=-1)
    out  = einsum('bhnm,bhmd->bhnd', attn, v) -> merge heads -> @ w_out + b_out

Shapes: x [4, 2048, 512], dist [4, 8, 2048, 2048], w_qkv [512, 1536],
w_out [512, 512], b_out [512].

Sharding over 8 cores: core m handles batch m//2, heads 4*(m%2) .. +4.
Each core computes its 4 heads' attention plus the partial out-projection
for its batch; host sums the two partials per batch and adds b_out.

Device layout (head-PAIR stacking on the partition axis):
 - scores are computed TRANSPOSED: S^T [keys(part), queries(free)] so the
   attn@v matmul contracts keys on the partition dim with no transposes.
 - q^T/k^T for a head PAIR are stacked on partitions: rows 0-63 = head 2p,
   rows 64-127 = head 2p+1. The two heads' QK^T matmuls are K=64 row-tiles
   at array rows (0,0)/(64,0) and run CONCURRENTLY in the PE; the q/k
   projections become full M=128 matmuls; the out-projection contracts the
   stacked pair K=128 (PSUM accumulates across pairs).
 - dist is folded in as exp(dots) = exp(qk)*exp(dist): host precomputes
   exp(dist^T) in bf16 and the DVE multiplies it into exp(scores) — except
   for the PE_KB key blocks, where raw dist^T is shipped instead and the
   PE adds it via identity-matmul accumulation. The PE share keeps the
   tensor engine dense enough that HAM holds K=8/8; ACT (exp) is the
   binding engine.
 - softmax skips the max-subtraction (scores are O(10)) and the
   denominator comes from augmenting v with a ones column (row 64 of the
   AV output).
 - everything is bf16 except PSUM accumulation and the normalization.
"""

import numpy as np

N_CORES = 8
B = 4
NTOK = 2048
DIM = 512
HEADS = 8
DH = 64  # head dim
NH = HEADS // 2  # heads per core (4)
NPAIR = NH // 2  # head pairs per core (2)
INNER = HEADS * DH
SCALE = DH ** -0.5
QC = 1024  # query chunk (free-dim) per attention psum block
NKB = NTOK // 128  # key blocks of 128
PE_KB = tuple(kb for kb in range(NKB) if kb % 3 == 0)


def _kb_on_pe(variant, kb):
    if "ped" in variant:
        return True
    if "exdo" in variant:
        return False
    return kb in PE_KB


def _build_nc(repeats=1, variant="mix3"):
    """variant flags (substring match):
      mix3  - default: dist on PE for PE_KB blocks, DVE multiply otherwise
      ped   - all dist on PE; exdo - all dist on DVE
      timing-only ablations (results wrong): nodma, nomul, noav.
    """
    import concourse.bacc as bacc
    import concourse.mybir as mybir
    import concourse.tile as tile
    from concourse.bass import ts
    from concourse.masks import make_identity

    f32 = mybir.dt.float32
    bf16 = mybir.dt.bfloat16
    Exp = mybir.ActivationFunctionType.Exp

    dbg = "dbg" in variant

    nc = bacc.Bacc("TRN2", target_bir_lowering=False, debug=False)

    xT_d = nc.dram_tensor("xT", [DIM, NTOK], bf16, kind="ExternalInput").ap()
    wq_d = nc.dram_tensor("wq", [DIM, NH * DH], bf16, kind="ExternalInput").ap()
    wk_d = nc.dram_tensor("wk", [DIM, NH * DH], bf16, kind="ExternalInput").ap()
    wv_d = nc.dram_tensor("wv", [DIM, NH * DH], bf16, kind="ExternalInput").ap()
    # per-head key-block rows hold either exp(dist^T) (DVE path) or raw
    # dist^T (PE path) — see make_in_maps
    ed_d = nc.dram_tensor("expdT", [NH, NTOK, NTOK], bf16, kind="ExternalInput").ap()
    wo_d = nc.dram_tensor("wo", [NH * DH, DIM], bf16, kind="ExternalInput").ap()
    part_d = nc.dram_tensor("part", [NTOK, DIM], f32, kind="ExternalOutput").ap()
    if dbg:
        qT2_o = nc.dram_tensor("qT2_o", [128, NPAIR, NTOK], bf16, kind="ExternalOutput").ap()
        kT2_o = nc.dram_tensor("kT2_o", [128, NPAIR, NTOK], bf16, kind="ExternalOutput").ap()
        v_o = nc.dram_tensor("v_o", [128, NH, NKB, DH + 1], bf16, kind="ExternalOutput").ap()
        ex_o = nc.dram_tensor("ex_o", [128, QC], bf16, kind="ExternalOutput").ap()
        em_o = nc.dram_tensor("em_o", [128, QC], bf16, kind="ExternalOutput").ap()
        po_o = nc.dram_tensor("po_o", [DH + 1, QC], f32, kind="ExternalOutput").ap()
        rec_o = nc.dram_tensor("rec_o", [1, QC], f32, kind="ExternalOutput").ap()
        oT2_o = nc.dram_tensor("oT2_o", [128, QC], bf16, kind="ExternalOutput").ap()

    with tile.TileContext(nc) as tc:
        for _rep in range(repeats):
            with (
                tc.tile_pool(name="consts", bufs=1) as consts,
                tc.tile_pool(name="qkv", bufs=1) as qkv,
            ):
                xT_sb = consts.tile([128, DIM // 128, NTOK], bf16)
                nc.sync.dma_start(xT_sb[:], xT_d.rearrange("(c p) n -> p c n", p=128))
                wq_sb = consts.tile([128, DIM // 128, NH * DH], bf16)
                nc.sync.dma_start(wq_sb[:], wq_d.rearrange("(c p) n -> p c n", p=128))
                wk_sb = consts.tile([128, DIM // 128, NH * DH], bf16)
                nc.sync.dma_start(wk_sb[:], wk_d.rearrange("(c p) n -> p c n", p=128))
                wv_sb = consts.tile([128, DIM // 128, NH * DH], bf16)
                nc.sync.dma_start(wv_sb[:], wv_d.rearrange("(c p) n -> p c n", p=128))
                # w_out rows for the pair stacked on partitions: [128, pair, DIM]
                wo_sb = consts.tile([128, NPAIR, DIM], bf16)
                nc.sync.dma_start(wo_sb[:], wo_d.rearrange("(p q) n -> q p n", q=128))

                ident32 = consts.tile([128, 128], f32)
                make_identity(nc, ident32)
                ident = consts.tile([128, 128], bf16)
                nc.scalar.copy(ident[:], ident32[:])

                # head pair p stacked on partitions: rows 0-63 head 2p, 64-127 head 2p+1
                qT2 = qkv.tile([128, NPAIR, NTOK], bf16)
                kT2 = qkv.tile([128, NPAIR, NTOK], bf16)
                v_sb = qkv.tile([128, NH, NKB, DH + 1], bf16)
                ones32 = consts.tile([128, NH, NKB, 1], f32)
                nc.gpsimd.memset(ones32[:], 1.0)
                nc.scalar.copy(v_sb[:, :, :, DH : DH + 1], ones32[:])

                # ---- phase 1: projections (k, v, q-half0; q-half1 is
                # interleaved into qc0's attention loop) ----
                with (
                    tc.tile_pool(name="p1qk", bufs=2, space="PSUM") as p1qk,
                    tc.tile_pool(name="p1v", bufs=2, space="PSUM") as p1v,
                ):
                    def qk_proj(dst, w_sb, p, half, pool, tag=""):
                        kw = {"tag": tag} if tag else {}
                        ps = pool.tile([128, QC], f32, name="ps", **kw)
                        for c in range(DIM // 128):
                            for j in range(QC // 512):
                                nc.tensor.matmul(
                                    ps[:, ts(j, 512)],
                                    w_sb[:, c, ts(p, 128)],
                                    xT_sb[:, c, half * QC + 512 * j : half * QC + 512 * (j + 1)],
                                    start=(c == 0),
                                    stop=(c == DIM // 128 - 1),
                                )
                        if tag:
                            # interleaved into attention: ACT is saturated there
                            nc.vector.tensor_copy(dst[:, p, ts(half, QC)], ps[:])
                        else:
                            nc.scalar.copy(dst[:, p, ts(half, QC)], ps[:])

                    for p in range(NPAIR):
                        qk_proj(kT2, wk_sb, p, 0, p1qk)
                        qk_proj(kT2, wk_sb, p, 1, p1qk)
                    for i in range(NKB):
                        ps_v = p1v.tile([128, NH * DH], f32)
                        for c in range(DIM // 128):
                            nc.tensor.matmul(
                                ps_v[:],
                                xT_sb[:, c, ts(i, 128)],
                                wv_sb[:, c, :],
                                start=(c == 0),
                                stop=(c == DIM // 128 - 1),
                            )
                        nc.scalar.copy(
                            v_sb[:, :, i, 0:DH],
                            ps_v.rearrange("p (h d) -> p h d", h=NH),
                        )
                    for p in range(NPAIR):
                        qk_proj(qT2, wq_sb, p, 0, p1qk)

                # ---- phase 2+3: attention + out-projection ----
                with (
                    tc.tile_pool(name="spsum", bufs=2, space="PSUM") as spsum,
                    tc.tile_pool(name="opsum", bufs=2, space="PSUM") as opsum,
                    tc.tile_pool(name="distp", bufs=10) as distp,
                    tc.tile_pool(name="expp", bufs=10) as expp,
                    tc.tile_pool(name="emp", bufs=8) as emp,
                    tc.tile_pool(name="otp", bufs=4) as otp,
                    tc.tile_pool(name="pocp", bufs=4) as pocp,
                    tc.tile_pool(name="smalls", bufs=4) as smalls,
                    tc.tile_pool(name="outp", bufs=3) as outp,
                ):
                    def out_proj(oT2_src, qc_src, i, act_evac=False):
                        # pair-stacked K=128, accumulate pairs in PSUM
                        pp = spsum.tile([128, QC], f32, name="pp", tag="ps")
                        for p in range(NPAIR):
                            nc.tensor.matmul(
                                pp[:, 0:DIM],
                                oT2_src[p][:, ts(i, 128)],
                                wo_sb[:, p, :],
                                start=(p == 0),
                                stop=(p == NPAIR - 1),
                            )
                        ob = outp.tile([128, DIM], f32, name="ob")
                        if act_evac:
                            nc.scalar.copy(ob[:], pp[:, 0:DIM])
                        else:
                            nc.vector.tensor_copy(ob[:], pp[:, 0:DIM])
                        nc.sync.dma_start(
                            part_d[qc_src * QC + i * 128 : qc_src * QC + (i + 1) * 128, :],
                            ob[:],
                        )

                    prev_oT2 = None
                    for qc in range(NTOK // QC):
                        oT2 = [otp.tile([128, QC], bf16, name="oT2") for _ in range(NPAIR)]
                        for p in range(NPAIR):
                            po = [opsum.tile([DH + 1, QC], f32, name="po") for _ in range(2)]
                            for kb in range(NKB):
                                ed_t = []
                                for s in range(2):
                                    t = distp.tile([128, QC], bf16, name="ed")
                                    if "nodma" not in variant:
                                        nc.sync.dma_start(
                                            t[:], ed_d[2 * p + s, ts(kb, 128), ts(qc, QC)]
                                        )
                                    ed_t.append(t)
                                on_pe = _kb_on_pe(variant, kb) and "nomul" not in variant
                                ps = [spsum.tile([128, QC], f32, name="ps") for _ in range(2)]
                                # two K=64 row-tiles (rows 0-63 / 64-127), issued
                                # alternating (a,b,a,b): adjacent MMs target
                                # disjoint row groups and run concurrently, and
                                # each LDWEIGHTS hits rows the streaming MM
                                # doesn't occupy
                                for s, j in ((0, 0), (1, 0), (0, 1), (1, 1)):
                                    pb = 64 * s
                                    nc.tensor.matmul(
                                        ps[s][:, ts(j, 512)],
                                        kT2[pb : pb + 64, p, ts(kb, 128)],
                                        qT2[pb : pb + 64, p, qc * QC + 512 * j : qc * QC + 512 * (j + 1)],
                                        start=True,
                                        stop=not on_pe,
                                    )
                                if on_pe:
                                    for s in range(2):
                                        for j in range(QC // 512):
                                            nc.tensor.matmul(
                                                ps[s][:, ts(j, 512)],
                                                ident[:],
                                                ed_t[s][:, ts(j, 512)],
                                                start=False,
                                                stop=True,
                                            )
                                for s in range(2):
                                    ex = expp.tile([128, QC], bf16, name="ex")
                                    nc.scalar.activation(ex[:], ps[s][:], Exp)
                                    if on_pe or "nomul" in variant:
                                        em = ex
                                    else:
                                        em = emp.tile([128, QC], bf16, name="em")
                                        nc.vector.tensor_mul(em[:], ex[:], ed_t[s][:])
                                    if dbg and qc == 0 and p == 0 and kb == 0 and s == 0:
                                        nc.sync.dma_start(ex_o[:], ex[:])
                                        nc.sync.dma_start(em_o[:], em[:])
                                    if "noav" not in variant:
                                        for j in range(QC // 512):
                                            nc.tensor.matmul(
                                                po[s][:, ts(j, 512)],
                                                v_sb[:, 2 * p + s, kb, :],
                                                em[:, ts(j, 512)],
                                                start=(kb == 0),
                                                stop=(kb == NKB - 1),
                                            )
                                # previous chunk's out-projection, interleaved so
                                # the PE never idles at the qc boundary
                                if prev_oT2 is not None and p == 0 and kb % 2 == 1:
                                    out_proj(prev_oT2, qc - 1, kb // 2)
                                # qc0: interleave the deferred q-half1 projections
                                if qc == 0 and p == 0 and kb in (2, 6):
                                    qk_proj(qT2, wq_sb, kb // 4, 1, spsum, tag="ps")
                            # normalize (softmax denom = row 64): copy po out of
                            # PSUM first (frees the opsum ring for the next
                            # pair), then reciprocal/broadcast/scale from SBUF.
                            for s in range(2):
                                poc = pocp.tile([DH, QC], f32, name="poc")
                                nc.vector.tensor_copy(poc[:], po[s][0:DH, :])
                                den = smalls.tile([1, QC], f32, name="den", tag="den")
                                nc.vector.tensor_copy(den[:], po[s][DH : DH + 1, :])
                                # NOTE: reciprocal_approx_fast needs a
                                # base-partition-0 input (base-64 views return
                                # garbage), hence the den copy
                                rec = smalls.tile([1, QC], f32, name="rec", tag="rec")
                                nc.vector.reciprocal_approx_fast(rec[:], den[:])
                                rb = smalls.tile([DH, QC], f32, name="rb", tag="rb")
                                nc.gpsimd.partition_broadcast(rb[:], rec[:])
                                nc.vector.tensor_mul(
                                    oT2[p][64 * s : 64 * s + 64, :], poc[:], rb[:]
                                )
                                if dbg and qc == 0 and p == 0 and s == 0:
                                    nc.sync.dma_start(po_o[0:DH, :], poc[:])
                                    nc.sync.dma_start(po_o[DH : DH + 1, :], den[:])
                                    nc.sync.dma_start(rec_o[:], rec[:])
                            if dbg and qc == 0 and p == 0:
                                nc.sync.dma_start(oT2_o[:], oT2[0][:])
                        prev_oT2 = oT2
                    # last chunk's out-projection (tail; ACT is idle here)
                    for i in range(QC // 128):
                        out_proj(prev_oT2, NTOK // QC - 1, i, act_evac=True)
                    if dbg:
                        nc.sync.dma_start(qT2_o[:], qT2[:])
                        nc.sync.dma_start(kT2_o[:], kT2[:])
                        nc.sync.dma_start(v_o[:], v_sb[:])

    nc.compile()
    return nc


_NC_CACHE = {}


def _get_nc(repeats=1, variant=None):
    if variant is None:
        variant = KERNEL_VARIANT
    key = (repeats, variant)
    if key not in _NC_CACHE:
        _NC_CACHE[key] = _build_nc(repeats, variant)
    return _NC_CACHE[key]


def make_in_maps(x, dist, w_qkv, w_out, variant=None):
    """Host-side sharding: per-core input dicts (final device dtypes)."""
    import ml_dtypes

    if variant is None:
        variant = KERNEL_VARIANT
    bf16 = ml_dtypes.bfloat16
    x = np.asarray(x, dtype=np.float32)
    dist = np.asarray(dist, dtype=np.float32)
    w_qkv = np.asarray(w_qkv, dtype=np.float32)
    w_out = np.asarray(w_out, dtype=np.float32)
    in_maps = []
    for m in range(N_CORES):
        b = m // 2
        h0 = NH * (m % 2)
        wq = np.ascontiguousarray(w_qkv[:, h0 * DH : (h0 + NH) * DH]) * np.float32(SCALE)
        wk = np.ascontiguousarray(w_qkv[:, INNER + h0 * DH : INNER + (h0 + NH) * DH])
        wv = np.ascontiguousarray(w_qkv[:, 2 * INNER + h0 * DH : 2 * INNER + (h0 + NH) * DH])
        dT = np.ascontiguousarray(dist[b, h0 : h0 + NH].transpose(0, 2, 1))
        if "ped" in variant:
            ed = dT.astype(bf16)
        elif "exdo" in variant:
            ed = np.exp(dT).astype(bf16)
        else:
            # mixed: PE_KB key blocks carry raw dist (PE identity-add), the
            # rest carry exp(dist) (DVE multiply)
            ed = np.exp(dT)
            for kb in PE_KB:
                ed[:, kb * 128 : (kb + 1) * 128, :] = dT[:, kb * 128 : (kb + 1) * 128, :]
            ed = ed.astype(bf16)
        in_maps.append(
            {
                "xT": np.ascontiguousarray(x[b].T).astype(bf16),
                "wq": wq.astype(bf16),
                "wk": wk.astype(bf16),
                "wv": wv.astype(bf16),
                "expdT": ed,
                "wo": np.ascontiguousarray(w_out[h0 * DH : (h0 + NH) * DH, :]).astype(bf16),
            }
        )
    return in_maps


def cast_in_maps(nc, in_maps):
    """Compat shim: make_in_maps already emits final dtypes."""
    return in_maps


def assemble(results, b_out):
    """Sum the two per-batch partials and add bias."""
    out = np.empty((B, NTOK, DIM), dtype=np.float32)
    for b in range(B):
        out[b] = results[2 * b]["part"] + results[2 * b + 1]["part"] + b_out
    return out


KERNEL_VARIANT = "mix3"


def kernel(x, dist, w_qkv, w_out, b_out):
    from concourse.bass_utils import run_bass_kernel_spmd

    nc = _get_nc(variant=KERNEL_VARIANT)
    in_maps = make_in_maps(x, dist, w_qkv, w_out)
    res = run_bass_kernel_spmd(nc, in_maps, core_ids=list(range(N_CORES)))
    return assemble(res.results, np.asarray(b_out, dtype=np.float32))
